# revision 1
# baseline (speedup 1.0000x reference)
"""Trainium2 Bass kernel for sparse-in -> dense-hidden -> sampled-out net.

  val1 = relu(in_values @ W1.T[active_in_indices] + b1)        # [B, H]
  val2 = einsum('bh,bkh->bk', val1, W2[active_label_indices]) + b2[active_label_indices]

Strategy: data-parallel over batch across 8 NeuronCores. Each core holds the
full W1.T / W2 tables in its HBM and performs large indirect-DMA row gathers
(4096 x 512B rows per sample in one instruction), then computes dots on DVE.
"""

import numpy as np

B, NNZ, F_DIM, H, C, KOUT = 128, 128, 135909, 128, 670091, 4096
N_CORES = 8
BPC = B // N_CORES          # samples per core
CHUNKS = KOUT // 128        # 128-row chunks per sample

_CACHE = {}


def build_program(nnz=NNZ, f_dim=F_DIM, h=H, c=C, kout=KOUT, bpc=BPC):
    import concourse.bass as bass
    import concourse.bacc as bacc
    import concourse.mybir as mybir
    import concourse.tile as tile

    fp32 = mybir.dt.float32
    i32 = mybir.dt.int32
    chunks = kout // 128

    nc = bacc.Bacc("TRN2", target_bir_lowering=False, debug=False)

    w1t = nc.dram_tensor("w1t", [f_dim, h], fp32, kind="ExternalInput")
    w2 = nc.dram_tensor("w2", [c, h], fp32, kind="ExternalInput")
    invt = nc.dram_tensor("invt", [nnz, bpc], fp32, kind="ExternalInput")
    idx1t = nc.dram_tensor("idx1t", [nnz, bpc], i32, kind="ExternalInput")
    idx2t = nc.dram_tensor("idx2t", [128, bpc * chunks], i32, kind="ExternalInput")
    b2gt = nc.dram_tensor("b2gt", [128, bpc * chunks], fp32, kind="ExternalInput")
    b1r = nc.dram_tensor("b1r", [1, h], fp32, kind="ExternalInput")
    out = nc.dram_tensor("val2", [bpc, kout], fp32, kind="ExternalOutput")

    with tile.TileContext(nc) as tc:
        with (
            tc.tile_pool(name="const", bufs=1) as cpool,
            tc.tile_pool(name="g1s", bufs=3) as g1spool,
            tc.tile_pool(name="g2", bufs=3) as g2pool,
            tc.tile_pool(name="tmp", bufs=3) as tmppool,
            tc.tile_pool(name="small", bufs=3) as spool,
            tc.tile_pool(name="psum", bufs=1, space="PSUM") as psum,
            tc.tile_pool(name="psumb", bufs=2, space="PSUM") as psumb,
        ):
            # memset order matters: PE warm-up waits on the LAST memset's
            # tick, observing all three.
            ones_row = cpool.tile([1, 128], fp32)
            nc.gpsimd.memset(ones_row[:], 1.0)
            ones_col = cpool.tile([128, 1], fp32)
            nc.gpsimd.memset(ones_col[:], 1.0)
            warm_ps = psum.tile([1, 1], fp32)
            nc.tensor.matmul(
                warm_ps[:], lhsT=ones_col[:], rhs=ones_col[:], start=True, stop=True
            )

            invt_t = cpool.tile([nnz, bpc], fp32)
            nc.gpsimd.dma_start(out=invt_t[:], in_=invt[:, :])
            idx1_t = cpool.tile([nnz, bpc], i32)
            nc.gpsimd.dma_start(out=idx1_t[:], in_=idx1t[:, :])
            idx2_t = cpool.tile([128, bpc * chunks], i32)
            nc.gpsimd.dma_start(out=idx2_t[:], in_=idx2t[:, :])
            b2g_t = cpool.tile([128, bpc * chunks], fp32)
            nc.gpsimd.dma_start(out=b2g_t[:], in_=b2gt[:, :])
            b1_t = cpool.tile([1, h], fp32)
            nc.gpsimd.dma_start(out=b1_t[:], in_=b1r[:, :])

            # ---- stage 1: hidden layer ----
            # one [128,1]-index gather per sample (HW indirect DMA consumes
            # one index per partition, fetching a contiguous row each)
            v1_ps = psum.tile([1, bpc * h], fp32)
            for j in range(bpc):
                g1 = g1spool.tile([nnz, h], fp32, tag="g1")
                nc.gpsimd.indirect_dma_start(
                    out=g1[:],
                    out_offset=None,
                    in_=w1t[:, :],
                    in_offset=bass.IndirectOffsetOnAxis(
                        ap=idx1_t[:, j : j + 1], axis=0
                    ),
                )
                g1s = g1spool.tile([nnz, h], fp32, tag="g1s")
                nc.vector.tensor_scalar_mul(
                    g1s[:], g1[:], invt_t[:, j : j + 1]
                )
                nc.tensor.matmul(
                    v1_ps[:, j * h : (j + 1) * h],
                    lhsT=ones_col[:],
                    rhs=g1s[:],
                    start=True,
                    stop=True,
                )
            v1tmp = cpool.tile([1, bpc * h], fp32)
            nc.vector.tensor_tensor(
                out=v1tmp[:],
                in0=v1_ps[:],
                in1=b1_t[:]
                .rearrange("p (o h) -> p o h", o=1)
                .to_broadcast([1, bpc, h]),
                op=mybir.AluOpType.add,
            )
            v1flat = cpool.tile([1, bpc * h], fp32)
            nc.vector.tensor_scalar_max(v1flat[:], v1tmp[:], 0.0)

            # ---- stage 2: sampled output layer ----
            val2_all = cpool.tile([128, bpc * chunks], fp32)
            for j in range(bpc):
                g2 = g2pool.tile([128, kout], fp32)
                for cix in range(chunks):
                    nc.gpsimd.indirect_dma_start(
                        out=g2[:, cix * h : (cix + 1) * h],
                        out_offset=None,
                        in_=w2[:, :],
                        in_offset=bass.IndirectOffsetOnAxis(
                            ap=idx2_t[:, j * chunks + cix : j * chunks + cix + 1],
                            axis=0,
                        ),
                    )
                bc_ps = psumb.tile([128, h], fp32)
                nc.tensor.matmul(
                    bc_ps[:],
                    lhsT=ones_row[:],
                    rhs=v1flat[:, j * h : (j + 1) * h],
                    start=True,
                    stop=True,
                )
                tmp = tmppool.tile([128, kout], fp32)
                nc.vector.tensor_tensor(
                    out=tmp[:],
                    in0=g2[:],
                    in1=bc_ps[:]
                    .rearrange("p (o h) -> p o h", o=1)
                    .to_broadcast([128, chunks, h]),
                    op=mybir.AluOpType.mult,
                )
                red = spool.tile([128, chunks], fp32)
                nc.vector.tensor_reduce(
                    out=red[:],
                    in_=tmp[:].rearrange("p (c h) -> p c h", c=chunks),
                    axis=mybir.AxisListType.X,
                    op=mybir.AluOpType.add,
                )
                nc.vector.tensor_add(
                    out=val2_all[:, j * chunks : (j + 1) * chunks],
                    in0=red[:],
                    in1=b2g_t[:, j * chunks : (j + 1) * chunks],
                )
            nc.sync.dma_start(
                out=out.ap().rearrange("j (p c) -> p j c", p=128),
                in_=val2_all[:].rearrange("p (j c) -> p j c", j=bpc),
            )
    nc.finalize()
    return nc


def make_core_inputs(in_values, active_in_indices, active_label_indices, W1T, b1, b2,
                     nnz=NNZ, h=H, kout=KOUT, bpc=BPC, n_cores=N_CORES):
    """Build the per-core input maps (host-side sharding + layout transforms)."""
    chunks = kout // 128
    b1r = np.ascontiguousarray(b1.reshape(1, h))
    in_maps = []
    for cid in range(n_cores):
        s = slice(cid * bpc, (cid + 1) * bpc)
        ali = active_label_indices[s]                      # [bpc, kout]
        # [p, j*chunks + c] = ali[j, p*chunks + c]
        idx2t = np.ascontiguousarray(
            ali.reshape(bpc, 128, chunks).transpose(1, 0, 2).reshape(128, bpc * chunks)
        )
        b2g = b2[ali]                                      # [bpc, kout]
        b2gt = np.ascontiguousarray(
            b2g.reshape(bpc, 128, chunks).transpose(1, 0, 2).reshape(128, bpc * chunks)
        ).astype(np.float32)
        in_maps.append(
            {
                "w1t": W1T,
                "w2": None,  # filled by caller (shared ref)
                "invt": np.ascontiguousarray(in_values[s].T),
                "idx1t": np.ascontiguousarray(active_in_indices[s].T),
                "idx2t": idx2t,
                "b2gt": b2gt,
                "b1r": b1r,
            }
        )
    return in_maps


def kernel(in_values, active_in_indices, active_label_indices, W1, b1, W2, b2):
    from concourse.bass_utils import run_bass_kernel_spmd

    in_values = np.asarray(in_values, dtype=np.float32)
    active_in_indices = np.asarray(active_in_indices, dtype=np.int32)
    active_label_indices = np.asarray(active_label_indices, dtype=np.int32)
    W1 = np.asarray(W1, dtype=np.float32)
    b1 = np.asarray(b1, dtype=np.float32)
    W2 = np.asarray(W2, dtype=np.float32)
    b2 = np.asarray(b2, dtype=np.float32)

    if "nc" not in _CACHE:
        _CACHE["nc"] = build_program()
    nc = _CACHE["nc"]

    W1T = np.ascontiguousarray(W1.T)
    in_maps = make_core_inputs(
        in_values, active_in_indices, active_label_indices, W1T, b1, b2
    )
    for m in in_maps:
        m["w2"] = W2

    res = run_bass_kernel_spmd(nc, in_maps, list(range(N_CORES)))
    val2 = np.concatenate([r["val2"] for r in res.results], axis=0)
    return val2, active_label_indices

